# revision 18
# baseline (speedup 1.0000x reference)
"""Trainium2 Bass kernel for nn_FOGCNConv (GNN message passing).

Math (reference):
    weight = softmax(importance, axis=0)            # [C, F]
    edge_score = cnt @ weight                       # [E, F]
    msgs = embedding[src] * edge_score              # [E, F]
    new_embedding = segment_sum(msgs, dst, N)       # [N, F]
    node_score = segment_sum(edge_score, dst, N)    # [N, F]
    out = new_embedding / node_score

Key structural facts (hardcoded; guaranteed by the input spec):
    N=20000 nodes, E=640000 edges, C=64, F=128, and dst is a permutation of
    arange(E) % N  =>  every node has exactly DEG=32 incoming edges.

Strategy:
  - Host: sort edges by dst. Then the two segment-sums become perfectly
    regular reductions over groups of 32 consecutive edges.
  - Shard *contiguous dst ranges* across the 8 cores (2500 nodes / 80000
    edges each) => fully independent cores, no collectives.
  - Device (per core, per 128-node window = 4096 edges):
      * dma_gather embedding rows for the window's src indices
        (edge e -> partition e%128, free block e//128)
      * edge_score tiles [128e,128f] via PE matmul (lhsT = cnt^T tile,
        rhs = softmax weights)
      * msgs = gathered * edge_score on DVE
      * new_embedding window [128f, 128n] via PE "segment matmul": for each
        edge tile t, out[:, 4t:4t+4] = msgs_t^T @ P where P[e, j] =
        (e//32 == j) is a constant one-hot (edges are dst-sorted).
      * node_score factored: segsum(cnt) via DVE strided reduce, then one
        matmul with the softmax weights: ns^T = W^T @ segsum_cnt.
      * out window = new_embedding * 1/node_score (DVE), F-major.
  - cnt is fed pre-transposed and "half-packed" to [128, ...] so DMA uses
    all 128 partitions: partition (half*64 + c) holds cnt^T[c] for the
    window's half-th group of 2048 edges; the softmax weights are
    replicated across both partition halves to keep matmul base partitions
    aligned.
"""

import sys

if "/opt/trn_rl_repo" not in sys.path:
    sys.path.insert(0, "/opt/trn_rl_repo")

import numpy as np

# Problem sizes (fixed by the spec).
N_NODES = 20000
N_EDGES = 640000
C = 64
F = 128
N_CORES = 8
NPC = N_NODES // N_CORES       # 2500 nodes per core
EPC = N_EDGES // N_CORES       # 80000 edges per core
DEG = N_EDGES // N_NODES       # 32 edges per node (dst is a permutation of arange(E)%N)
WIN_NODES = 128                # nodes per window
EPW = WIN_NODES * DEG          # 4096 edges per window
N_WIN = -(-NPC // WIN_NODES)   # 20 windows per core (last partial: 68 nodes)
PAD_EPC = N_WIN * EPW          # 81920 padded edges per core
HALF = EPW // 2                # 2048
GATHER_CHUNK = 1024            # max idxs per dma_gather (SWDGE ring capacity)

_CACHE = {}


def _build_nc(n_win=N_WIN, skip=()):
    import concourse.bass as bass  # noqa: F401
    import concourse.bacc as bacc
    import concourse.bass_isa as bass_isa
    import concourse.tile as tile
    import concourse.mybir as mybir
    from contextlib import ExitStack

    f32 = mybir.dt.float32
    i16 = mybir.dt.int16
    AF = mybir.ActivationFunctionType
    AX = mybir.AxisListType

    nc = bacc.Bacc("TRN2", target_bir_lowering=False, debug=False)
    cntp = nc.declare_dram_parameter("cntp", [128, N_WIN * HALF], f32, isOutput=False)
    idx = nc.declare_dram_parameter("idx", [128, PAD_EPC // 16], i16, isOutput=False)
    emb = nc.declare_dram_parameter("emb", [N_NODES, F], f32, isOutput=False)
    imp = nc.declare_dram_parameter("imp", [C, F], f32, isOutput=False)
    pbase = nc.declare_dram_parameter("pbase", [128, 4], f32, isOutput=False)
    out = nc.declare_dram_parameter("out", [F, NPC], f32, isOutput=True)

    with ExitStack() as ctx:
        tc = ctx.enter_context(tile.TileContext(nc))
        const = ctx.enter_context(tc.tile_pool(name="const", bufs=1))

        # ---- constants ----
        pbase_sb = const.tile([128, 4], f32)
        nc.sync.dma_start(pbase_sb[:], pbase[:, :])
        idx_sb = const.tile([128, PAD_EPC // 16], i16)
        nc.sync.dma_start(idx_sb[:], idx[:, :])

        # ---- softmax(importance, axis=0) -> w2 (replicated in both halves) ----
        # No PE/PSUM here: the partition-axis sum is done by DVE 32x32 block
        # transposes + a free-axis reduce, so every PSUM bank below is touched
        # by matmuls of a single PE row position only (base partition 0 pools
        # vs base partition 64 pools). Mixing row positions within one PSUM
        # bank crashes the device. (gpsimd.partition_all_reduce also crashes
        # the device on this runtime - don't use it.)
        imp_sb = const.tile([C, F], f32)
        nc.sync.dma_start(imp_sb[:], imp[:, :])
        exp_sb = const.tile([C, F], f32)
        nc.scalar.activation(exp_sb[:], imp_sb[:], AF.Exp)
        expT_sb = const.tile([128, C], f32)
        for i in range(C // 32):
            for j in range(F // 32):
                nc.vector.transpose(
                    expT_sb[32 * j:32 * j + 32, 32 * i:32 * i + 32],
                    exp_sb[32 * i:32 * i + 32, 32 * j:32 * j + 32])
        s_sb = const.tile([128, 1], f32)
        nc.vector.reduce_sum(s_sb[:], expT_sb[:], axis=AX.X)
        rec_sb = const.tile([128, 1], f32)
        nc.vector.reciprocal(rec_sb[:], s_sb[:])
        wT_sb = const.tile([128, C], f32)
        nc.vector.tensor_scalar_mul(wT_sb[:], expT_sb[:], rec_sb[:])
        w2_sb = const.tile([128, F], f32)
        for i in range(F // 32):
            for j in range(C // 32):
                nc.vector.transpose(
                    w2_sb[32 * j:32 * j + 32, 32 * i:32 * i + 32],
                    wT_sb[32 * i:32 * i + 32, 32 * j:32 * j + 32])
        nc.sync.dma_start(w2_sb[C:128, :], w2_sb[0:C, :])

        cnt_pool = ctx.enter_context(tc.tile_pool(name="cnt", bufs=2))
        gath_pool = ctx.enter_context(tc.tile_pool(name="gath", bufs=2))
        es_pool = ctx.enter_context(tc.tile_pool(name="es", bufs=2, space="PSUM"))
        msgs_pool = ctx.enter_context(tc.tile_pool(name="msgs", bufs=3))
        ne_pool = ctx.enter_context(tc.tile_pool(name="ne", bufs=2, space="PSUM"))
        ns_pool = ctx.enter_context(tc.tile_pool(name="ns", bufs=1, space="PSUM"))
        red_pool = ctx.enter_context(tc.tile_pool(name="red", bufs=2))
        rns_pool = ctx.enter_context(tc.tile_pool(name="rns", bufs=2))

        out_sb = const.tile([128, NPC], f32)

        # ---- main loop over 128-node windows ----
        for w in range(n_win):
            nodes_w = min(WIN_NODES, NPC - w * WIN_NODES)
            nt = (nodes_w * DEG) // 128      # edge tiles this window (32 or 17)
            n_idx = nt * 128

            cnt_sb = cnt_pool.tile([128, HALF], f32, tag="cnt")
            nc.sync.dma_start(cnt_sb[:], cntp[:, w * HALF:(w + 1) * HALF])

            gath = gath_pool.tile([128, EPW], f32, tag="gath")
            if "gather" in skip:
                nc.vector.memset(gath[:, :nt * F], 1.0)
            else:
                # SWDGE descriptor ring holds dynamic_dma_scratch_size/16
                # descriptors; one gather descriptor per index. Chunk to
                # stay under the ring capacity (a bigger gather crashes
                # the device).
                gath3 = gath[:, :nt * F].rearrange("p (t f) -> p t f", f=F)
                for e0 in range(0, n_idx, GATHER_CHUNK):
                    ecnt = min(GATHER_CHUNK, n_idx - e0)
                    nc.gpsimd.dma_gather(
                        out_ap=gath3[:, e0 // 128:(e0 + ecnt) // 128, :],
                        in_ap=emb[:, :],
                        idxs_ap=idx_sb[:, w * (EPW // 16) + e0 // 16:
                                       w * (EPW // 16) + (e0 + ecnt) // 16],
                        num_idxs=ecnt,
                        num_idxs_reg=ecnt,
                        elem_size=F,
                    )

            ne_ps = ne_pool.tile([128, 128], f32, tag="ne")
            nb = -(-nt // 4)
            for b in range(nb):
                bt = min(4, nt - 4 * b)
                half = (4 * b) // 16      # batches never straddle halves (16%4==0)
                es_ps = es_pool.tile([128, 512], f32,
                                     tag="es_hi" if half else "es_lo")
                for j in range(bt):
                    t = 4 * b + j
                    tl = t % 16
                    lhsT = cnt_sb[64 * half:64 * half + 64, 128 * tl:128 * tl + 128]
                    rhs = w2_sb[64 * half:64 * half + 64, :]
                    nc.tensor.matmul(
                        es_ps[:, 128 * j:128 * (j + 1)], lhsT, rhs,
                        start=True, stop=True,
                    )
                msgs = msgs_pool.tile([128, 512], f32, tag="msgs")
                nc.vector.tensor_mul(
                    msgs[:, :128 * bt],
                    gath[:, 512 * b:512 * b + 128 * bt],
                    es_ps[:, :128 * bt],
                )
                for j in range(bt):
                    t = 4 * b + j
                    nc.tensor.matmul(
                        ne_ps[:, 4 * t:4 * t + 4],
                        msgs[:, 128 * j:128 * (j + 1)],
                        pbase_sb[:],
                        start=True, stop=True,
                    )

            # node_score path: segsum(cnt) on DVE, then ns^T = W^T @ segsum.
            red_sb = red_pool.tile([128, 64], f32, tag="red")
            cnt3 = cnt_sb[:].rearrange("p (g d) -> p g d", d=DEG)
            nc.vector.reduce_sum(red_sb[:], cnt3, axis=AX.X)
            lo_w = min(64, nodes_w)
            hi_w = nodes_w - lo_w
            ns_lo = ns_pool.tile([128, 64], f32, tag="ns_lo")
            nc.tensor.matmul(ns_lo[:], w2_sb[0:64, :], red_sb[0:64, :],
                             start=True, stop=True)
            rns_sb = rns_pool.tile([128, 128], f32, tag="rns")
            nc.vector.reciprocal(rns_sb[:, :lo_w], ns_lo[:, :lo_w])
            if hi_w > 0:
                ns_hi = ns_pool.tile([128, 64], f32, tag="ns_hi")
                nc.tensor.matmul(ns_hi[:], w2_sb[64:128, :], red_sb[64:128, :],
                                 start=True, stop=True)
                nc.vector.reciprocal(rns_sb[:, 64:64 + hi_w], ns_hi[:, :hi_w])
            nc.vector.tensor_mul(
                out_sb[:, w * WIN_NODES:w * WIN_NODES + nodes_w],
                ne_ps[:, :nodes_w],
                rns_sb[:, :nodes_w],
            )

        nc.sync.dma_start(out[:, :], out_sb[:, :NPC])

    nc.compile()
    return nc


def get_nc():
    if "nc" not in _CACHE:
        _CACHE["nc"] = _build_nc()
    return _CACHE["nc"]


def prep_in_maps(inputs):
    cnt = np.asarray(inputs["cnt"], dtype=np.float32)
    emb = np.ascontiguousarray(np.asarray(inputs["embedding"], dtype=np.float32))
    imp = np.ascontiguousarray(np.asarray(inputs["importance"], dtype=np.float32))
    src = np.asarray(inputs["src"], dtype=np.int64)
    dst = np.asarray(inputs["dst"], dtype=np.int64)

    perm = np.argsort(dst, kind="stable")
    src_s = src[perm]
    cnt_s = cnt[perm]

    pbase = np.zeros((128, 4), np.float32)
    pbase[np.arange(128), np.arange(128) // DEG] = 1.0

    in_maps = []
    for c in range(N_CORES):
        sl = slice(c * EPC, (c + 1) * EPC)
        cnt_core = np.zeros((PAD_EPC, C), np.float32)
        cnt_core[:EPC] = cnt_s[sl]
        src_core = np.zeros((PAD_EPC,), np.int64)
        src_core[:EPC] = src_s[sl]
        # half-pack: [w, half, j, c] -> [half*64+c, w*HALF+j]
        cc = cnt_core.reshape(N_WIN, 2, HALF, C)
        cntp = np.ascontiguousarray(
            cc.transpose(1, 3, 0, 2).reshape(128, N_WIN * HALF))
        # wrapped int16 index layout: idx i at [i%16, i//16], replicated x8
        idxw = np.ascontiguousarray(
            np.tile(src_core.reshape(PAD_EPC // 16, 16).T, (8, 1)).astype(np.int16))
        in_maps.append({
            "cntp": cntp,
            "idx": idxw,
            "emb": emb,
            "imp": imp,
            "pbase": pbase,
        })
    return in_maps


def unshard(core_outs):
    # each core out: [F, NPC] (F-major); concat over node axis, transpose.
    full = np.concatenate(core_outs, axis=1)          # [F, N]
    return np.ascontiguousarray(full.T.astype(np.float32))


def run(inputs, trace=False):
    from concourse.bass_utils import run_bass_kernel_spmd

    nc = get_nc()
    in_maps = prep_in_maps(inputs)
    res = run_bass_kernel_spmd(
        nc, in_maps, core_ids=list(range(N_CORES)), trace=trace)
    outs = [res.results[i]["out"] for i in range(N_CORES)]
    return unshard(outs), res


def kernel(**inputs):
    out, _ = run(inputs, trace=False)
    return out


# revision 19
# speedup vs baseline: 1.8979x; 1.8979x over previous
"""Trainium2 Bass kernel for nn_FOGCNConv (GNN message passing).

Math (reference):
    weight = softmax(importance, axis=0)            # [C, F]
    edge_score = cnt @ weight                       # [E, F]
    msgs = embedding[src] * edge_score              # [E, F]
    new_embedding = segment_sum(msgs, dst, N)       # [N, F]
    node_score = segment_sum(edge_score, dst, N)    # [N, F]
    out = new_embedding / node_score

Key structural facts (hardcoded; guaranteed by the input spec):
    N=20000 nodes, E=640000 edges, C=64, F=128, and dst is a permutation of
    arange(E) % N  =>  every node has exactly DEG=32 incoming edges.

Strategy:
  - Host: sort edges by dst. Then the two segment-sums become perfectly
    regular reductions over groups of 32 consecutive edges.
  - Shard *contiguous dst ranges* across the 8 cores (2500 nodes / 80000
    edges each) => fully independent cores, no collectives.
  - Device (per core, per 128-node window = 4096 edges = 32 edge tiles):
      * dma_gather embedding rows for the window's src indices
        (edge e -> partition e%128, free block e//128), 1024 idxs per
        gather (SWDGE ring capacity), rotated over 4 SWDGE queues.
      * edge_score for tile pair (j, j+16) in ONE K=128 fp16 matmul:
        lhsT = packed cnt^T [128, 128] (c-halves stacked), rhs = Wstack
        [128, 256] block-diagonal softmax weights -> es [128e, 256] f32.
      * msgs = gathered * edge_score on DVE (one 3D-strided op per pair),
        cast to fp16.
      * new_embedding window [128f, 128n] via PE "segment matmuls": per
        edge tile t, out[:, 4t:4t+4] = msgs_t^T @ P where P[e, j] =
        (e//32 == j) is a constant one-hot (edges are dst-sorted).
      * node_score factored: segsum(cnt) via DVE strided reduce ->
        block-diagonal fp16 redx, then ONE matmul ns^T = w2^T @ redx.
      * out window = new_embedding * 1/node_score (DVE), F-major.
  - cnt is fed pre-transposed fp16 and "half-packed" to [128, ...] so DMA
    uses all 128 partitions: partition (half*64 + c) holds cnt^T[c] for
    the window's half-th group of 2048 edges.
  - PE-row-position rule: a PSUM bank must only ever be written by matmuls
    whose stationary operand sits at one SBUF base partition. All matmuls
    here use base partition 0. (Mixing positions in one bank crashes the
    device; so does gpsimd.partition_all_reduce, and so does a dma_gather
    of more than ring-capacity indices.)
"""

import sys

if "/opt/trn_rl_repo" not in sys.path:
    sys.path.insert(0, "/opt/trn_rl_repo")

import numpy as np

# Problem sizes (fixed by the spec).
N_NODES = 20000
N_EDGES = 640000
C = 64
F = 128
N_CORES = 8
NPC = N_NODES // N_CORES       # 2500 nodes per core
EPC = N_EDGES // N_CORES       # 80000 edges per core
DEG = N_EDGES // N_NODES       # 32 edges per node
WIN_NODES = 128                # nodes per window
EPW = WIN_NODES * DEG          # 4096 edges per window
N_WIN = -(-NPC // WIN_NODES)   # 20 windows per core (last partial: 68 nodes)
PAD_EPC = N_WIN * EPW          # 81920 padded edges per core
HALF = EPW // 2                # 2048
GATHER_CHUNK = 1024            # max idxs per dma_gather (SWDGE ring capacity)
N_QUEUES = 4                   # SWDGE queues; rotate gathers across them

_CACHE = {}


def _build_nc(n_win=N_WIN, skip=()):
    import concourse.bass as bass  # noqa: F401
    import concourse.bacc as bacc
    import concourse.tile as tile
    import concourse.mybir as mybir
    from contextlib import ExitStack

    f32 = mybir.dt.float32
    f16 = mybir.dt.float16
    i16 = mybir.dt.int16
    AF = mybir.ActivationFunctionType
    AX = mybir.AxisListType

    nc = bacc.Bacc("TRN2", target_bir_lowering=False, debug=False,
                   num_swdge_queues=N_QUEUES)
    cntp = nc.declare_dram_parameter("cntp", [128, N_WIN * HALF], f16, isOutput=False)
    idx = nc.declare_dram_parameter("idx", [128, PAD_EPC // 16], i16, isOutput=False)
    emb = nc.declare_dram_parameter("emb", [N_NODES, F], f32, isOutput=False)
    imp = nc.declare_dram_parameter("imp", [C, F], f32, isOutput=False)
    pbase = nc.declare_dram_parameter("pbase", [128, 4], f16, isOutput=False)
    out = nc.declare_dram_parameter("out", [F, NPC], f32, isOutput=True)

    with ExitStack() as ctx:
        tc = ctx.enter_context(tile.TileContext(nc))
        const = ctx.enter_context(tc.tile_pool(name="const", bufs=1))

        # ---- constants ----
        pbase_sb = const.tile([128, 4], f16)
        nc.sync.dma_start(pbase_sb[:], pbase[:, :])
        idx_sb = const.tile([128, PAD_EPC // 16], i16)
        nc.sync.dma_start(idx_sb[:], idx[:, :])

        # ---- softmax(importance, axis=0) on DVE (block transposes) ----
        imp_sb = const.tile([C, F], f32)
        nc.sync.dma_start(imp_sb[:], imp[:, :])
        exp_sb = const.tile([C, F], f32)
        nc.scalar.activation(exp_sb[:], imp_sb[:], AF.Exp)
        expT_sb = const.tile([128, C], f32)
        for i in range(C // 32):
            for j in range(F // 32):
                nc.vector.transpose(
                    expT_sb[32 * j:32 * j + 32, 32 * i:32 * i + 32],
                    exp_sb[32 * i:32 * i + 32, 32 * j:32 * j + 32])
        s_sb = const.tile([128, 1], f32)
        nc.vector.reduce_sum(s_sb[:], expT_sb[:], axis=AX.X)
        rec_sb = const.tile([128, 1], f32)
        nc.vector.reciprocal(rec_sb[:], s_sb[:])
        wT_sb = const.tile([128, C], f32)
        nc.vector.tensor_scalar_mul(wT_sb[:], expT_sb[:], rec_sb[:])
        w2_sb = const.tile([128, F], f32)
        for i in range(F // 32):
            for j in range(C // 32):
                nc.vector.transpose(
                    w2_sb[32 * j:32 * j + 32, 32 * i:32 * i + 32],
                    wT_sb[32 * i:32 * i + 32, 32 * j:32 * j + 32])
        nc.sync.dma_start(w2_sb[C:128, :], w2_sb[0:C, :])

        # fp16 weight forms: w2_16 (W stacked twice) and block-diag Wstack.
        w2_16 = const.tile([128, F], f16)
        nc.vector.tensor_copy(w2_16[:], w2_sb[:])
        wstack = const.tile([128, 2 * F], f16)
        nc.vector.memset(wstack[:], 0.0)
        nc.vector.tensor_copy(wstack[0:C, 0:F], w2_sb[0:C, :])
        nc.vector.tensor_copy(wstack[C:128, F:2 * F], w2_sb[C:128, :])

        out_sb = const.tile([128, NPC], f32)

        cnt_pool = ctx.enter_context(tc.tile_pool(name="cnt", bufs=2))
        gath_pool = ctx.enter_context(tc.tile_pool(name="gath", bufs=2))
        es_pool = ctx.enter_context(tc.tile_pool(name="es", bufs=3, space="PSUM"))
        msgs_pool = ctx.enter_context(tc.tile_pool(name="msgs", bufs=3))
        ne_pool = ctx.enter_context(tc.tile_pool(name="ne", bufs=2, space="PSUM"))
        ns_pool = ctx.enter_context(tc.tile_pool(name="ns", bufs=2, space="PSUM"))
        red_pool = ctx.enter_context(tc.tile_pool(name="red", bufs=2))
        redx_pool = ctx.enter_context(tc.tile_pool(name="redx", bufs=2))
        rns_pool = ctx.enter_context(tc.tile_pool(name="rns", bufs=2))

        gq = 0  # rotating SWDGE queue index

        # ---- main loop over 128-node windows ----
        # Edges are padded to whole windows with cnt=0 / idx=0, so every
        # window runs the full 32 tiles; only the final column copies are
        # restricted to the window's real node count.
        for w in range(n_win):
            nodes_w = min(WIN_NODES, NPC - w * WIN_NODES)

            cnt_sb = cnt_pool.tile([128, HALF], f16, tag="cnt")
            nc.sync.dma_start(cnt_sb[:], cntp[:, w * HALF:(w + 1) * HALF])

            gath = gath_pool.tile([128, EPW], f32, tag="gath")
            if "gather" in skip:
                nc.vector.memset(gath[:], 1.0)
            else:
                gath3 = gath[:].rearrange("p (t f) -> p t f", f=F)
                for e0 in range(0, EPW, GATHER_CHUNK):
                    nc.gpsimd.dma_gather(
                        out_ap=gath3[:, e0 // 128:(e0 + GATHER_CHUNK) // 128, :],
                        in_ap=emb[:, :],
                        idxs_ap=idx_sb[:, w * (EPW // 16) + e0 // 16:
                                       w * (EPW // 16) + (e0 + GATHER_CHUNK) // 16],
                        num_idxs=GATHER_CHUNK,
                        num_idxs_reg=GATHER_CHUNK,
                        elem_size=F,
                        queue_num=gq,
                    )
                    gq = (gq + 1) % N_QUEUES

            ne_ps = ne_pool.tile([128, 128], f32, tag="ne")
            for j in range(16):  # tile pair (j, j+16)
                es_ps = es_pool.tile([128, 256], f32, tag="es")
                nc.tensor.matmul(
                    es_ps[:], cnt_sb[:, 128 * j:128 * (j + 1)], wstack[:],
                    start=True, stop=True,
                )
                msgs = msgs_pool.tile([128, 256], f16, tag="msgs")
                g3 = gath[:].rearrange("p (t f) -> p t f", f=F)
                nc.vector.tensor_mul(
                    msgs[:].rearrange("p (t f) -> p t f", f=F),
                    g3[:, j:j + 17:16, :],
                    es_ps[:].rearrange("p (t f) -> p t f", f=F),
                )
                nc.tensor.matmul(
                    ne_ps[:, 4 * j:4 * j + 4],
                    msgs[:, 0:F], pbase_sb[:],
                    start=True, stop=True,
                )
                nc.tensor.matmul(
                    ne_ps[:, 64 + 4 * j:64 + 4 * j + 4],
                    msgs[:, F:2 * F], pbase_sb[:],
                    start=True, stop=True,
                )

            # node_score path: segsum(cnt) -> block-diag redx -> one matmul.
            red_sb = red_pool.tile([128, 64], f32, tag="red")
            cnt3 = cnt_sb[:].rearrange("p (g d) -> p g d", d=DEG)
            nc.vector.reduce_sum(red_sb[:], cnt3, axis=AX.X)
            redx_sb = redx_pool.tile([128, 128], f16, tag="redx")
            nc.vector.memset(redx_sb[:], 0.0)
            nc.vector.tensor_copy(redx_sb[0:64, 0:64], red_sb[0:64, :])
            nc.vector.tensor_copy(redx_sb[64:128, 64:128], red_sb[64:128, :])
            ns_ps = ns_pool.tile([128, 128], f32, tag="ns")
            nc.tensor.matmul(ns_ps[:], w2_16[:], redx_sb[:],
                             start=True, stop=True)

            rns_sb = rns_pool.tile([128, 128], f32, tag="rns")
            nc.vector.reciprocal(rns_sb[:, :nodes_w], ns_ps[:, :nodes_w])
            nc.vector.tensor_mul(
                out_sb[:, w * WIN_NODES:w * WIN_NODES + nodes_w],
                ne_ps[:, :nodes_w],
                rns_sb[:, :nodes_w],
            )

        nc.sync.dma_start(out[:, :], out_sb[:, :NPC])

    nc.compile()
    return nc


def get_nc():
    if "nc" not in _CACHE:
        _CACHE["nc"] = _build_nc()
    return _CACHE["nc"]


def prep_in_maps(inputs):
    cnt = np.asarray(inputs["cnt"], dtype=np.float32)
    emb = np.ascontiguousarray(np.asarray(inputs["embedding"], dtype=np.float32))
    imp = np.ascontiguousarray(np.asarray(inputs["importance"], dtype=np.float32))
    src = np.asarray(inputs["src"], dtype=np.int64)
    dst = np.asarray(inputs["dst"], dtype=np.int64)

    perm = np.argsort(dst, kind="stable")
    src_s = src[perm]
    cnt_s = cnt[perm].astype(np.float16)

    pbase = np.zeros((128, 4), np.float16)
    pbase[np.arange(128), np.arange(128) // DEG] = 1.0

    in_maps = []
    for c in range(N_CORES):
        sl = slice(c * EPC, (c + 1) * EPC)
        cnt_core = np.zeros((PAD_EPC, C), np.float16)
        cnt_core[:EPC] = cnt_s[sl]
        src_core = np.zeros((PAD_EPC,), np.int64)
        src_core[:EPC] = src_s[sl]
        # half-pack: [w, half, j, c] -> [half*64+c, w*HALF+j]
        cc = cnt_core.reshape(N_WIN, 2, HALF, C)
        cntp = np.ascontiguousarray(
            cc.transpose(1, 3, 0, 2).reshape(128, N_WIN * HALF))
        # wrapped int16 index layout: idx i at [i%16, i//16], replicated x8
        idxw = np.ascontiguousarray(
            np.tile(src_core.reshape(PAD_EPC // 16, 16).T, (8, 1)).astype(np.int16))
        in_maps.append({
            "cntp": cntp,
            "idx": idxw,
            "emb": emb,
            "imp": imp,
            "pbase": pbase,
        })
    return in_maps


def unshard(core_outs):
    # each core out: [F, NPC] (F-major); concat over node axis, transpose.
    full = np.concatenate(core_outs, axis=1)          # [F, N]
    return np.ascontiguousarray(full.T.astype(np.float32))


def run(inputs, trace=False):
    from concourse.bass_utils import run_bass_kernel_spmd

    nc = get_nc()
    in_maps = prep_in_maps(inputs)
    res = run_bass_kernel_spmd(
        nc, in_maps, core_ids=list(range(N_CORES)), trace=trace)
    outs = [res.results[i]["out"] for i in range(N_CORES)]
    return unshard(outs), res


def kernel(**inputs):
    out, _ = run(inputs, trace=False)
    return out


# revision 22
# speedup vs baseline: 2.7304x; 1.4387x over previous
"""Trainium2 Bass kernel for nn_FOGCNConv (GNN message passing).

Math (reference):
    weight = softmax(importance, axis=0)            # [C, F]
    edge_score = cnt @ weight                       # [E, F]
    msgs = embedding[src] * edge_score              # [E, F]
    new_embedding = segment_sum(msgs, dst, N)       # [N, F]
    node_score = segment_sum(edge_score, dst, N)    # [N, F]
    out = new_embedding / node_score

Key structural facts (hardcoded; guaranteed by the input spec):
    N=20000 nodes, E=640000 edges, C=64, F=128, and dst is a permutation of
    arange(E) % N  =>  every node has exactly DEG=32 incoming edges.

Strategy:
  - Host: sort edges by dst. Then the two segment-sums become perfectly
    regular reductions over groups of 32 consecutive edges.
  - Shard *contiguous dst ranges* across the 8 cores (2500 nodes / 80000
    edges each) => fully independent cores, no collectives.
  - Device (per core, per 128-node window = 4096 edges = 32 edge tiles):
      * dma_gather embedding rows for the window's src indices
        (edge e -> partition e%128, free block e//128), 1024 idxs per
        gather (SWDGE ring capacity), rotated over 4 SWDGE queues.
      * edge_score for tile pair (j, j+16) in ONE K=128 fp16 matmul:
        lhsT = packed cnt^T [128, 128] (c-halves stacked), rhs = Wstack
        [128, 256] block-diagonal softmax weights -> es [128e, 256] f32.
      * msgs = gathered * edge_score on DVE (one 3D-strided op per pair),
        cast to fp16.
      * new_embedding window [128f, 128n] via PE "segment matmuls": per
        edge tile t, out[:, 4t:4t+4] = msgs_t^T @ P where P[e, j] =
        (e//32 == j) is a constant one-hot (edges are dst-sorted).
      * node_score factored: segsum(cnt) via DVE strided reduce ->
        block-diagonal fp16 redx, then ONE matmul ns^T = w2^T @ redx.
      * out window = new_embedding * 1/node_score (DVE), F-major.
  - cnt is fed pre-transposed fp16 and "half-packed" to [128, ...] so DMA
    uses all 128 partitions: partition (half*64 + c) holds cnt^T[c] for
    the window's half-th group of 2048 edges.
  - PE-row-position rule: a PSUM bank must only ever be written by matmuls
    whose stationary operand sits at one SBUF base partition. All matmuls
    here use base partition 0. (Mixing positions in one bank crashes the
    device; so does gpsimd.partition_all_reduce, and so does a dma_gather
    of more than ring-capacity indices.)
"""

import sys

if "/opt/trn_rl_repo" not in sys.path:
    sys.path.insert(0, "/opt/trn_rl_repo")

import numpy as np

# Problem sizes (fixed by the spec).
N_NODES = 20000
N_EDGES = 640000
C = 64
F = 128
N_CORES = 8
NPC = N_NODES // N_CORES       # 2500 nodes per core
EPC = N_EDGES // N_CORES       # 80000 edges per core
DEG = N_EDGES // N_NODES       # 32 edges per node
WIN_NODES = 128                # nodes per window
EPW = WIN_NODES * DEG          # 4096 edges per window
N_WIN = -(-NPC // WIN_NODES)   # 20 windows per core (last partial: 68 nodes)
PAD_EPC = N_WIN * EPW          # 81920 padded edges per core
HALF = EPW // 2                # 2048
GATHER_CHUNK = 1024            # max idxs per dma_gather (SWDGE ring capacity)
N_QUEUES = 4                   # SWDGE queues; rotate gathers across them

_CACHE = {}


def _build_nc(n_win=N_WIN, skip=()):
    import concourse.bass as bass  # noqa: F401
    import concourse.bacc as bacc
    import concourse.tile as tile
    import concourse.mybir as mybir
    from contextlib import ExitStack

    f32 = mybir.dt.float32
    f16 = mybir.dt.float16
    i16 = mybir.dt.int16
    AF = mybir.ActivationFunctionType
    AX = mybir.AxisListType

    nc = bacc.Bacc("TRN2", target_bir_lowering=False, debug=False,
                   num_swdge_queues=N_QUEUES)
    cntp = nc.declare_dram_parameter("cntp", [128, N_WIN * HALF], f16, isOutput=False)
    idx = nc.declare_dram_parameter("idx", [128, PAD_EPC // 16], i16, isOutput=False)
    emb = nc.declare_dram_parameter("emb", [N_NODES, F], f32, isOutput=False)
    imp = nc.declare_dram_parameter("imp", [C, F], f32, isOutput=False)
    pbase = nc.declare_dram_parameter("pbase", [128, 4], f16, isOutput=False)
    out = nc.declare_dram_parameter("out", [F, NPC], f32, isOutput=True)

    with ExitStack() as ctx:
        tc = ctx.enter_context(tile.TileContext(nc))
        const = ctx.enter_context(tc.tile_pool(name="const", bufs=1))

        # ---- constants ----
        pbase_sb = const.tile([128, 4], f16)
        nc.sync.dma_start(pbase_sb[:], pbase[:, :])
        idx_sb = const.tile([128, PAD_EPC // 16], i16)
        nc.sync.dma_start(idx_sb[:], idx[:, :])

        # ---- softmax(importance, axis=0) on DVE (block transposes) ----
        imp_sb = const.tile([C, F], f32)
        nc.sync.dma_start(imp_sb[:], imp[:, :])
        exp_sb = const.tile([C, F], f32)
        nc.scalar.activation(exp_sb[:], imp_sb[:], AF.Exp)
        expT_sb = const.tile([128, C], f32)
        for i in range(C // 32):
            for j in range(F // 32):
                nc.vector.transpose(
                    expT_sb[32 * j:32 * j + 32, 32 * i:32 * i + 32],
                    exp_sb[32 * i:32 * i + 32, 32 * j:32 * j + 32])
        s_sb = const.tile([128, 1], f32)
        nc.vector.reduce_sum(s_sb[:], expT_sb[:], axis=AX.X)
        rec_sb = const.tile([128, 1], f32)
        nc.vector.reciprocal(rec_sb[:], s_sb[:])
        wT_sb = const.tile([128, C], f32)
        nc.vector.tensor_scalar_mul(wT_sb[:], expT_sb[:], rec_sb[:])
        w2_sb = const.tile([128, F], f32)
        for i in range(F // 32):
            for j in range(C // 32):
                nc.vector.transpose(
                    w2_sb[32 * j:32 * j + 32, 32 * i:32 * i + 32],
                    wT_sb[32 * i:32 * i + 32, 32 * j:32 * j + 32])
        nc.sync.dma_start(w2_sb[C:128, :], w2_sb[0:C, :])

        # fp16 block-diagonal Wstack for the paired edge-score matmuls.
        wstack = const.tile([128, 2 * F], f16)
        nc.vector.memset(wstack[:], 0.0)
        nc.vector.tensor_copy(wstack[0:C, 0:F], w2_sb[0:C, :])
        nc.vector.tensor_copy(wstack[C:128, F:2 * F], w2_sb[C:128, :])

        out_sb = const.tile([128, NPC], f32)

        cnt_pool = ctx.enter_context(tc.tile_pool(name="cnt", bufs=2))
        gath_pool = ctx.enter_context(tc.tile_pool(name="gath", bufs=3))
        es_pool = ctx.enter_context(tc.tile_pool(name="es", bufs=3, space="PSUM"))
        msgs_pool = ctx.enter_context(tc.tile_pool(name="msgs", bufs=3))
        ne_pool = ctx.enter_context(tc.tile_pool(name="ne", bufs=2, space="PSUM"))
        ns_pool = ctx.enter_context(tc.tile_pool(name="ns", bufs=1, space="PSUM"))
        red_pool = ctx.enter_context(tc.tile_pool(name="red", bufs=2))
        rns_pool = ctx.enter_context(tc.tile_pool(name="rns", bufs=2))

        gq = 0  # rotating SWDGE queue index

        # ---- main loop over 128-node windows ----
        # Edges are padded to whole windows with cnt=0 / idx=0, so every
        # window runs the full 32 tiles; only the final column copies are
        # restricted to the window's real node count.
        for w in range(n_win):
            nodes_w = min(WIN_NODES, NPC - w * WIN_NODES)

            cnt_sb = cnt_pool.tile([128, HALF], f16, tag="cnt")
            nc.sync.dma_start(cnt_sb[:], cntp[:, w * HALF:(w + 1) * HALF])

            gath = gath_pool.tile([128, EPW], f32, tag="gath")
            if "gather" in skip:
                nc.vector.memset(gath[:], 1.0)
            else:
                gath3 = gath[:].rearrange("p (t f) -> p t f", f=F)
                for e0 in range(0, EPW, GATHER_CHUNK):
                    nc.gpsimd.dma_gather(
                        out_ap=gath3[:, e0 // 128:(e0 + GATHER_CHUNK) // 128, :],
                        in_ap=emb[:, :],
                        idxs_ap=idx_sb[:, w * (EPW // 16) + e0 // 16:
                                       w * (EPW // 16) + (e0 + GATHER_CHUNK) // 16],
                        num_idxs=GATHER_CHUNK,
                        num_idxs_reg=GATHER_CHUNK,
                        elem_size=F,
                        queue_num=gq,
                    )
                    gq = (gq + 1) % N_QUEUES

            ne_ps = ne_pool.tile([128, 128], f32, tag="ne")
            for j in range(16):  # tile pair (j, j+16)
                es_ps = es_pool.tile([128, 256], f32, tag="es")
                nc.tensor.matmul(
                    es_ps[:], cnt_sb[:, 128 * j:128 * (j + 1)], wstack[:],
                    start=True, stop=True,
                )
                msgs = msgs_pool.tile([128, 256], f16, tag="msgs")
                g3 = gath[:].rearrange("p (t f) -> p t f", f=F)
                nc.vector.tensor_mul(
                    msgs[:].rearrange("p (t f) -> p t f", f=F),
                    g3[:, j:j + 17:16, :],
                    es_ps[:].rearrange("p (t f) -> p t f", f=F),
                )
                nc.tensor.matmul(
                    ne_ps[:, 4 * j:4 * j + 4],
                    msgs[:, 0:F], pbase_sb[:],
                    start=True, stop=True,
                )
                nc.tensor.matmul(
                    ne_ps[:, 64 + 4 * j:64 + 4 * j + 4],
                    msgs[:, F:2 * F], pbase_sb[:],
                    start=True, stop=True,
                )

            # node_score path: segsum(cnt) on DVE, then ns^T = W^T @ segsum.
            # Two f32 matmuls in position-dedicated PSUM banks (lo: PE rows
            # 0-63, hi: rows 64-127) to honor the PE-row-position rule.
            red_sb = red_pool.tile([128, 64], f32, tag="red")
            cnt3 = cnt_sb[:].rearrange("p (g d) -> p g d", d=DEG)
            nc.vector.reduce_sum(red_sb[:], cnt3, axis=AX.X)
            lo_w = min(64, nodes_w)
            hi_w = nodes_w - lo_w
            ns_lo = ns_pool.tile([128, 64], f32, tag="ns_lo")
            nc.tensor.matmul(ns_lo[:], w2_sb[0:64, :], red_sb[0:64, :],
                             start=True, stop=True)
            rns_sb = rns_pool.tile([128, 128], f32, tag="rns")
            nc.vector.reciprocal(rns_sb[:, :lo_w], ns_lo[:, :lo_w])
            if hi_w > 0:
                ns_hi = ns_pool.tile([128, 64], f32, tag="ns_hi")
                nc.tensor.matmul(ns_hi[:], w2_sb[64:128, :], red_sb[64:128, :],
                                 start=True, stop=True)
                nc.vector.reciprocal(rns_sb[:, 64:64 + hi_w], ns_hi[:, :hi_w])
            nc.vector.tensor_mul(
                out_sb[:, w * WIN_NODES:w * WIN_NODES + nodes_w],
                ne_ps[:, :nodes_w],
                rns_sb[:, :nodes_w],
            )

        nc.sync.dma_start(out[:, :], out_sb[:, :NPC])

    nc.compile()
    return nc


def get_nc():
    if "nc" not in _CACHE:
        _CACHE["nc"] = _build_nc()
    return _CACHE["nc"]


def prep_in_maps(inputs):
    cnt = np.asarray(inputs["cnt"], dtype=np.float32)
    emb = np.ascontiguousarray(np.asarray(inputs["embedding"], dtype=np.float32))
    imp = np.ascontiguousarray(np.asarray(inputs["importance"], dtype=np.float32))
    src = np.asarray(inputs["src"], dtype=np.int64)
    dst = np.asarray(inputs["dst"], dtype=np.int64)

    perm = np.argsort(dst, kind="stable")
    src_s = src[perm]
    cnt_s = cnt[perm].astype(np.float16)

    pbase = np.zeros((128, 4), np.float16)
    pbase[np.arange(128), np.arange(128) // DEG] = 1.0

    in_maps = []
    for c in range(N_CORES):
        sl = slice(c * EPC, (c + 1) * EPC)
        cnt_core = np.zeros((PAD_EPC, C), np.float16)
        cnt_core[:EPC] = cnt_s[sl]
        src_core = np.zeros((PAD_EPC,), np.int64)
        src_core[:EPC] = src_s[sl]
        # half-pack: [w, half, j, c] -> [half*64+c, w*HALF+j]
        cc = cnt_core.reshape(N_WIN, 2, HALF, C)
        cntp = np.ascontiguousarray(
            cc.transpose(1, 3, 0, 2).reshape(128, N_WIN * HALF))
        # wrapped int16 index layout: idx i at [i%16, i//16], replicated x8
        idxw = np.ascontiguousarray(
            np.tile(src_core.reshape(PAD_EPC // 16, 16).T, (8, 1)).astype(np.int16))
        in_maps.append({
            "cntp": cntp,
            "idx": idxw,
            "emb": emb,
            "imp": imp,
            "pbase": pbase,
        })
    return in_maps


def unshard(core_outs):
    # each core out: [F, NPC] (F-major); concat over node axis, transpose.
    full = np.concatenate(core_outs, axis=1)          # [F, N]
    return np.ascontiguousarray(full.T.astype(np.float32))


def run(inputs, trace=False):
    from concourse.bass_utils import run_bass_kernel_spmd

    nc = get_nc()
    in_maps = prep_in_maps(inputs)
    res = run_bass_kernel_spmd(
        nc, in_maps, core_ids=list(range(N_CORES)), trace=trace)
    outs = [res.results[i]["out"] for i in range(N_CORES)]
    return unshard(outs), res


def kernel(**inputs):
    out, _ = run(inputs, trace=False)
    return out
